# revision 64
# baseline (speedup 1.0000x reference)
"""MoE routing kernel (nn_JSMLP): per-row expert-indexed 3-layer MLP.

  out[n] = Wl[i] @ tanh(W2[i] @ tanh(W1[i] @ x[n] + b1[i]) + b2[i]) + bl[i],  i = ind[n]

Sharding strategy (hardcoded): expert-parallel across the 8 cores.
Host groups rows by expert (argsort of ind), pads each expert's rows to a
uniform capacity C, and assigns 32 consecutive experts to each core. Each
core then runs dense per-expert GEMMs in a transposed layout (hidden dim on
SBUF partitions, rows on the free dim), in bf16 with fp32 PSUM accumulation:

  L1: H1ᵀ[256, S] = W1augᵀ.T @ xaugᵀ      (bias via ones-row augmentation, K=65)
  L2: H2ᵀ[256, S] = W2ᵀ.T @ tanh(H1ᵀ)     (K=256 split in 2, bias seeded first
                                           via a tiny block-diagonal matmul)
  L3: outᵀ[64, S] = Wlᵀ.T @ tanh(H2ᵀ)     (two experts packed per 128-partition
                                           PSUM tile, bias as in L2)

Schedule notes (driven by the TimelineSim cost model):
- The DMA device is the binding resource (~19us of streaming at 360 B/ns).
  All loads are issued up front on one SP queue; stream segment b carries
  [xw(b+2), wb(b)], so a block's x/W1 land two segments before its W2/Wl and
  L2(b) is gated purely by its own wb arrival.
- PE executes in order, so emission software-pipelines blocks: iteration b
  emits L1(b+2), L2(b), L3(b-1), separated by scheduler-only no_sync fences
  that keep the list scheduler from hoisting a later iteration's tanh2-gated
  L3 into an earlier PE run (4-deep-wait-queue head-of-line stalls).
- The final two blocks are half-sized (2 experts) and their L3 runs in
  per-expert chunks, shrinking the serial post-stream chain
  L2->tanh2->L3->copy->store that dominates the tail; stores alternate
  SP/ACT HWDGE queues so a store blocked on its copy never delays the next
  store's descriptor generation.
- tanh runs on ScalarE over whole-block spans to amortize its fixed costs.

Everything is statically compiled after inspecting the routing (capacity C is
derived from the actual max expert load), so the single SPMD program is
identical across cores and only the per-core data differs.
"""

import numpy as np
import ml_dtypes

N, IN_DIM, H1, H2, LIN, NEXP = 16384, 64, 256, 256, 64, 256
NCORES = 8
EPC = NEXP // NCORES  # experts per core

BF16 = ml_dtypes.bfloat16

_cache = {}


def _plan(C):
    """Block sizes per core and derived layout offsets."""
    if C <= 128:
        sizes = [4] * 7 + [2, 2]
    else:
        sizes = [2] * 16
    assert sum(sizes) == EPC
    ncbs = [(s + 1) // 2 for s in sizes]
    e_off = np.concatenate([[0], np.cumsum(sizes)])
    o_off = np.concatenate([[0], np.cumsum([n * C for n in ncbs])])
    return sizes, ncbs, e_off, o_off


def _build_program(C):
    import concourse.bass as bass
    import concourse.tile as tile
    from concourse import bacc, mybir

    sizes, ncbs, e_off, o_off = _plan(C)
    blocks = len(sizes)
    Bmax = max(sizes)
    f32 = mybir.dt.float32
    bf16 = mybir.dt.bfloat16
    Tanh = mybir.ActivationFunctionType.Tanh

    O_WLC = blocks * 256            # ct sections: w2c | wlc | bdiag
    O_BD = O_WLC + blocks * 128
    CT = O_BD + Bmax * C
    TOT = int(o_off[-1])

    nc = bacc.Bacc("TRN2", target_bir_lowering=False, debug=False,
                   num_devices=NCORES)

    # xw slabs for all blocks live in one DRAM tensor (column-concatenated)
    # so the two-block prologue is a single DMA; w2+wl are merged per block
    # into one wb tensor.  Fewer DMAs keeps the serial HWDGE descriptor
    # generator (625ns per dma_start) well ahead of the transfers.
    xw_cols = [sizes[b] * (C + 256) for b in range(blocks)]
    xw_off = np.concatenate([[0], np.cumsum(xw_cols)])
    xw_d = nc.dram_tensor("xw", [65, int(xw_off[-1])], bf16,
                          kind="ExternalInput")
    wb_d = [nc.dram_tensor(f"wb{b}", [128, sizes[b] * 640], bf16,
                           kind="ExternalInput") for b in range(blocks)]
    ct_d = nc.dram_tensor("ct", [Bmax, CT], bf16, kind="ExternalInput")
    out_d = nc.dram_tensor("out", [128, TOT], bf16, kind="ExternalOutput")

    # store groups: pairs of blocks; with an odd block count the final store
    # covers just the last (small) block, minimizing the tail transfer
    sgroups = []
    b = 0
    while b < blocks:
        b2 = min(b + 2, blocks)
        sgroups.append((b, b2 - 1))
        b = b2
    store_after = {hi: lo for lo, hi in sgroups}

    with tile.TileContext(nc) as tc:
        with (
            tc.tile_pool(name="consts", bufs=1) as cpool,
            tc.tile_pool(name="wts", bufs=blocks) as wpool,
            tc.tile_pool(name="acts", bufs=8) as hpool,
            tc.tile_pool(name="ostage", bufs=1) as opool,
            tc.tile_pool(name="ph", bufs=3, space=bass.MemorySpace.PSUM) as phpool,
            tc.tile_pool(name="po", bufs=2, space=bass.MemorySpace.PSUM) as popool,
        ):
            ct = cpool.tile([Bmax, CT], bf16, tag="ct")
            w2ct = ct[:, 0:O_WLC]
            wlct = ct[0:2, O_WLC:O_BD]
            bdt = ct[:, O_BD:CT]
            # bf16 staging/store: halves output bytes on the serial DMA pipe;
            # the DVE copy below does the fp32->bf16 cast for free
            ostage = opool.tile([128, TOT], bf16, tag="out")

            # Stream order: segment b carries [xw(b+2), wb(b)] — xw runs two
            # segments ahead of its block's weights, so L1(b)+tanh1(b) are
            # long done when wb(b) lands and L2(b) is gated purely by its
            # own data arrival.  A single SP queue keeps HWDGE gens (2x625
            # per segment) well ahead of transfers (~2.3us per segment).
            xwts, wbts = {}, []

            def load_xw(b0, b1):
                xwt = wpool.tile([65, int(xw_off[b1] - xw_off[b0])], bf16,
                                 tag="xw", bufs=blocks, name=f"xwt{b0}",
                                 padded_shape=[65, 2 * Bmax * (C + 256)])
                nc.sync.dma_start(
                    xwt[:], xw_d.ap()[:, int(xw_off[b0]):int(xw_off[b1])])
                for b in range(b0, b1):
                    o = int(xw_off[b] - xw_off[b0])
                    xwts[b] = xwt[:, o:o + xw_cols[b]]

            load_xw(0, 2)
            for b in range(blocks):
                s = sizes[b]
                wbt = wpool.tile([128, s * 640], bf16, tag="wb",
                                 name=f"wbt{b}")
                nc.sync.dma_start(wbt[:], wb_d[b].ap())
                wbts.append(wbt)
                if b == 0:
                    # tiny ct load goes after wb(0): it is not needed until
                    # L2(0) and would otherwise delay wb(0)'s HWDGE gen
                    nc.sync.dma_start(ct[:], ct_d.ap())
                if b + 2 < blocks:
                    load_xw(b + 2, b + 3)

            h1s, h2s = {}, {}

            def stage_l1(b):
                s = sizes[b]
                S = s * C
                xwt = xwts[b]
                xgt = xwt[:, 0:S]
                w1tt = xwt[:, S:]
                # L1: H1preT — hidden half t at cols [t*512, t*512+S)
                # (512-aligned so no matmul output crosses a PSUM bank).
                ph1 = phpool.tile([128, 1024], f32, tag="ph", name=f"ph1_{b}")
                for j in range(s):
                    for t in range(2):
                        nc.tensor.matmul(
                            ph1[:, t * 512 + j * C : t * 512 + (j + 1) * C],
                            w1tt[:, j * 256 + t * 128 : j * 256 + (t + 1) * 128],
                            xgt[:, j * C : (j + 1) * C],
                        )
                h1 = hpool.tile([128, 2 * S], bf16, tag="h1", name=f"h1_{b}",
                                bufs=blocks, padded_shape=[128, 2 * Bmax * C])
                nc.scalar.activation(
                    h1[:].rearrange("p (t s) -> p t s", t=2),
                    ph1[:].rearrange("p (t s) -> p t s", t=2)[:, :, 0:S],
                    Tanh,
                )
                h1s[b] = h1

            def stage_l2(b):
                s = sizes[b]
                S = s * C
                wbt, h1 = wbts[b], h1s[b]
                w2at = wbt[:, 0:s * 256]
                w2bt = wbt[:, s * 256:s * 512]
                ph2 = phpool.tile([128, 1024], f32, tag="ph", name=f"ph2_{b}")
                h2 = hpool.tile([128, 2 * S], bf16, tag="h2", name=f"h2_{b}",
                                padded_shape=[128, 2 * Bmax * C])
                # bias seeded first over the span (hardware-proven
                # accumulation order), then 2 K-chunks per expert; the
                # per-iteration no_sync fences keep the always-ready bias
                # matmul from being hoisted into an earlier PE run.  The last
                # two (small) blocks run per-expert so their tanh2 chunks
                # pipeline with the remaining matmuls on the tail chain.
                chunks = [(0, s)]
                for (j0, j1) in chunks:
                    for t in range(2):
                        nc.tensor.matmul(
                            ph2[:, t * 512 + j0 * C : t * 512 + j1 * C],
                            w2ct[:, (b * 2 + t) * 128 : (b * 2 + t + 1) * 128],
                            bdt[:, j0 * C : j1 * C],
                            start=True, stop=False, skip_group_check=True,
                        )
                        for j in range(j0, j1):
                            nc.tensor.matmul(
                                ph2[:, t * 512 + j * C : t * 512 + (j + 1) * C],
                                w2at[:, j * 256 + t * 128 : j * 256 + (t + 1) * 128],
                                h1[:, j * C : (j + 1) * C],
                                start=False, stop=False, skip_group_check=True,
                            )
                            nc.tensor.matmul(
                                ph2[:, t * 512 + j * C : t * 512 + (j + 1) * C],
                                w2bt[:, j * 256 + t * 128 : j * 256 + (t + 1) * 128],
                                h1[:, S + j * C : S + (j + 1) * C],
                                start=False, stop=True, skip_group_check=True,
                            )
                    nc.scalar.activation(
                        h2[:].rearrange("p (t s) -> p t s", t=2)[:, :, j0 * C : j1 * C],
                        ph2[:].rearrange("p (t s) -> p t s", t=2)[:, :, j0 * C : j1 * C],
                        Tanh,
                    )
                h2s[b] = h2

            def stage_l3(b):
                s = sizes[b]
                S = s * C
                ncb = ncbs[b]
                wbt, h2 = wbts[b], h2s[b]
                wlat = wbt[:, s * 512:s * 576]
                wlbt = wbt[:, s * 576:s * 640]
                # experts packed 2-per-partition-block: expert j -> output
                # partitions [64*(j%2), +64), columns [(j//2)*C, +C)
                po = popool.tile([128, ncb * C], f32, tag="po", name=f"po_{b}",
                                 padded_shape=[128, ((Bmax + 1) // 2) * C])
                if b < blocks - 2:
                    for h in range(2):
                        nc.tensor.matmul(
                            po[h * 64 : (h + 1) * 64, 0 : ncb * C],
                            wlct[:, (b * 2 + h) * 64 : (b * 2 + h + 1) * 64],
                            bdt[0:2, 0 : ncb * C],
                            start=True, stop=False, skip_group_check=True,
                        )
                    for j in range(s):
                        h_, cb = j % 2, j // 2
                        nc.tensor.matmul(
                            po[h_ * 64 : (h_ + 1) * 64, cb * C : (cb + 1) * C],
                            wlat[:, j * 64 : (j + 1) * 64],
                            h2[:, j * C : (j + 1) * C],
                            start=False, stop=False, skip_group_check=True,
                        )
                        nc.tensor.matmul(
                            po[h_ * 64 : (h_ + 1) * 64, cb * C : (cb + 1) * C],
                            wlbt[:, j * 64 : (j + 1) * 64],
                            h2[:, S + j * C : S + (j + 1) * C],
                            start=False, stop=True, skip_group_check=True,
                        )
                else:
                    # last two (small) blocks: per-expert chunks, each an
                    # independent [bias, wla, wlb] group on its own
                    # 64-partition half, so L3 starts as soon as that
                    # expert's tanh2 chunk lands
                    for j in range(s):
                        h_, cb = j % 2, j // 2
                        nc.tensor.matmul(
                            po[h_ * 64 : (h_ + 1) * 64, cb * C : (cb + 1) * C],
                            wlct[:, (b * 2 + h_) * 64 : (b * 2 + h_ + 1) * 64],
                            bdt[0:2, cb * C : (cb + 1) * C],
                            start=True, stop=False, skip_group_check=True,
                        )
                        nc.tensor.matmul(
                            po[h_ * 64 : (h_ + 1) * 64, cb * C : (cb + 1) * C],
                            wlat[:, j * 64 : (j + 1) * 64],
                            h2[:, j * C : (j + 1) * C],
                            start=False, stop=False, skip_group_check=True,
                        )
                        nc.tensor.matmul(
                            po[h_ * 64 : (h_ + 1) * 64, cb * C : (cb + 1) * C],
                            wlbt[:, j * 64 : (j + 1) * 64],
                            h2[:, S + j * C : S + (j + 1) * C],
                            start=False, stop=True, skip_group_check=True,
                        )
                # a 1-expert block fills only the first 64-partition half;
                # copy just that half so the stale PSUM above it is never read
                prt = 64 if s == 1 else 128
                nc.vector.tensor_copy(
                    ostage[0:prt, o_off[b] : o_off[b] + ncb * C],
                    po[0:prt, 0 : ncb * C],
                )
                if b in store_after:
                    lo = store_after[b]
                    # stores alternate between the SP and ACT HWDGE queues so
                    # a store blocked on its copy (holding its queue's SEQ)
                    # never delays the NEXT store's descriptor generation;
                    # the final store uses SP (smallest DGE delay)
                    q = nc.sync if (b == blocks - 1 or lo % 4 == 0) else nc.scalar
                    q.dma_start(
                        out_d.ap()[:, o_off[lo] : o_off[b] + ncb * C],
                        ostage[:, o_off[lo] : o_off[b] + ncb * C],
                    )

            # scheduler-only fences (no runtime sems) between iterations keep
            # the list scheduler from hoisting a later iteration's tanh-gated
            # work into an earlier PE run, where it would head-of-line block
            # the 4-deep wait queue.  L3s of blocks whose wl streams in the
            # tail are emitted in a fenced epilogue, arrival-paced.
            # steady state: iteration b = [L1(b+2), L2(b), L3(b-2)].  L1
            # runs 2 iterations ahead (its xw streamed 2 segments early) so
            # tanh1(b) is long done when L2(b)'s data lands, and the shared
            # ph rotation always has ~1.5 iterations of WAR slack.
            # Scheduler-only fences (no runtime sems) keep the list
            # scheduler from hoisting a later iteration's tanh2-gated L3
            # into an earlier PE run, where it would head-of-line block the
            # 4-deep wait queue.  The tail iterations are left unfenced:
            # greedy readiness order is near-optimal there.
            stage_l1(0)
            stage_l1(1)
            tc.no_sync_barrier()
            # The last TAILN blocks' L3s are deferred to a fenced epilogue
            # AFTER every tail L2: the tail L2s (whose tanh2s gate the
            # critical chain) then issue back-to-back on PE, and the
            # deferred L3s execute inside the final tanh2 round trips.
            TAILN = 3
            for b in range(blocks):
                if b + 2 < blocks:
                    stage_l1(b + 2)
                if b == blocks - 1:
                    # final block's L2 is the critical tail chain: top
                    # priority so the list scheduler prefers it over the
                    # deferred L3 filler whenever both are ready
                    with tc.high_priority():
                        stage_l2(b)
                else:
                    stage_l2(b)
                if 1 <= b and b - 1 < blocks - TAILN:
                    stage_l3(b - 1)
                if b <= blocks - 2:
                    tc.no_sync_barrier()
            for k in range(blocks - TAILN, blocks):
                if k == blocks - 1:
                    with tc.high_priority():
                        stage_l3(k)
                else:
                    stage_l3(k)

    nc.compile()
    return nc


def _prep_inputs(x, ind, W1, b1, W2, b2, Wl, bl, C):
    """Group rows by expert and build the per-core padded device arrays."""
    sizes, ncbs, e_off, o_off = _plan(C)
    blocks = len(sizes)
    Bmax = max(sizes)
    O_WLC = blocks * 256
    O_BD = O_WLC + blocks * 128
    CT = O_BD + Bmax * C

    order = np.argsort(ind, kind="stable")
    counts = np.bincount(ind, minlength=NEXP)
    offs = np.zeros(NEXP + 1, np.int64)
    np.cumsum(counts, out=offs[1:])
    rows = [order[offs[e]:offs[e + 1]] for e in range(NEXP)]

    # Augmented, transposed weight tables (built once across all cores).
    w1aug = np.concatenate([W1, b1[:, :, None]], axis=2)  # [E, 256, 65]
    w2aug = np.concatenate([W2, b2[:, :, None]], axis=2)  # [E, 256, 257]
    wlaug = np.concatenate([Wl, bl[:, :, None]], axis=2)  # [E, 64, 257]

    xw_cols = [sizes[b] * (C + 256) for b in range(blocks)]
    xw_off = np.concatenate([[0], np.cumsum(xw_cols)])

    in_maps = []
    for k in range(NCORES):
        ct = np.zeros((Bmax, CT), np.float32)
        for j in range(Bmax):
            ct[j, O_BD + j * C : O_BD + (j + 1) * C] = 1.0
        m = {}
        xwall = np.zeros((65, int(xw_off[-1])), np.float32)
        for b in range(blocks):
            s = sizes[b]
            S = s * C
            xw = xwall[:, int(xw_off[b]):int(xw_off[b + 1])]
            xw[64, 0:S] = 1.0  # ones row of the augmented x
            wb = np.zeros((128, s * 640), np.float32)
            for j in range(s):
                e = k * EPC + int(e_off[b]) + j
                r = rows[e]
                xw[0:64, j * C : j * C + len(r)] = x[r].T
                xw[:, S + j * 256 : S + (j + 1) * 256] = w1aug[e].T
                wb[:, j * 256 : (j + 1) * 256] = w2aug[e, :, 0:128].T
                wb[:, s * 256 + j * 256 : s * 256 + (j + 1) * 256] = \
                    w2aug[e, :, 128:256].T
                wb[:, s * 512 + j * 64 : s * 512 + (j + 1) * 64] = \
                    wlaug[e, :, 0:128].T
                wb[:, s * 576 + j * 64 : s * 576 + (j + 1) * 64] = \
                    wlaug[e, :, 128:256].T
                ct[j, b * 256 : (b + 1) * 256] = w2aug[e, :, 256]
                ct[j // 2, O_WLC + b * 128 + (j % 2) * 64 :
                   O_WLC + b * 128 + (j % 2 + 1) * 64] = wlaug[e, :, 256]
            m[f"wb{b}"] = wb.astype(BF16)
        m["xw"] = xwall.astype(BF16)
        m["ct"] = ct.astype(BF16)
        in_maps.append(m)
    return in_maps, rows


def _unscatter(results, rows, C):
    sizes, ncbs, e_off, o_off = _plan(C)
    out = np.empty((N, LIN), np.float32)
    for k in range(NCORES):
        arr = np.asarray(results[k]["out"], np.float32)
        for b in range(len(sizes)):
            for j in range(sizes[b]):
                h, cb = j % 2, j // 2
                e = k * EPC + int(e_off[b]) + j
                r = rows[e]
                c0 = int(o_off[b]) + cb * C
                out[r, :] = arr[h * 64 : (h + 1) * 64, c0 : c0 + len(r)].T
    return out


def kernel(x, ind, W1, b1, W2, b2, Wl, bl):
    import os
    # defensive: recover cleanly if a previous run left a core wedged
    os.environ.setdefault("NEURON_RT_RESET_CORES", "1")
    from concourse.bass_utils import run_bass_kernel_spmd

    x = np.asarray(x, np.float32)
    ind = np.asarray(ind).astype(np.int64)
    W1 = np.asarray(W1, np.float32); b1 = np.asarray(b1, np.float32)
    W2 = np.asarray(W2, np.float32); b2 = np.asarray(b2, np.float32)
    Wl = np.asarray(Wl, np.float32); bl = np.asarray(bl, np.float32)

    counts = np.bincount(ind, minlength=NEXP)
    C = max(32, int(np.ceil(counts.max() / 32)) * 32)
    assert C <= 256, f"expert load {counts.max()} too imbalanced for this kernel"

    if C not in _cache:
        _cache[C] = _build_program(C)
    nc = _cache[C]

    in_maps, rows = _prep_inputs(x, ind, W1, b1, W2, b2, Wl, bl, C)
    res = run_bass_kernel_spmd(nc, in_maps, core_ids=list(range(NCORES)))
    return _unscatter(res.results, rows, C)
